# revision 42
# baseline (speedup 1.0000x reference)
"""Trainium2 Bass kernel: per-sample masked conv2d via 1-D Winograd F(2,3).

out[b] = conv2d(x[b], weight * m[b], stride=1, pad=1) + bias

Data parallel over batch (32 -> 8 cores x 4).  The 3x3 conv is decomposed
with 1-D Winograd F(2,3) along H (kw stays a shifted-matmul accumulation):

  per output row-pair t (28 tiles):   d_a = xpad[2t+a],  a = 0..3
    V0 = d0-d2, V1 = d1+d2, V2 = d2-d1, V3 = d1-d3          (input transform)
    U0 = g0, U1 = (g0+g1+g2)/2, U2 = (g0-g1+g2)/2, U3 = g2  (weight transform)
    M_j = sum_{kw,i} U_j  (x)kw  V_j                         (4 j-matmuls)
    out[2t]   = M0 + M1 + M2 + bias
    out[2t+1] = M1 - M2 - M3 + bias                          (inverse)

This trades 18 matmul-passes per output pixel for 12 (1.5x less PE time).
The 1/2 scales on U1/U2 are folded into the Activation-engine PSUM drains.
M is drained to bf16 so the DVE inverse runs in 2x mode; a final Act pass
applies bias while converting bf16 -> f32.  The matmul path stays f32r via
zero-copy bitcasts (f32r is an f32-bits dtype tag enabling the full-rate
PE mode).  Stationary tiles are 128-col slices of the U tiles (f32r slice
LDWEIGHTS is ldw-opt compatible; bf16 is not).
"""

import sys
from contextlib import ExitStack

for _p in ("/opt/trn_rl_repo",):
    if _p not in sys.path:
        sys.path.append(_p)

import ml_dtypes
import numpy as np

import concourse.bass as bass
import concourse.mybir as mybir
import concourse.tile as tile
from concourse import bacc, bass_utils

B, FIN, FOUT, KK, H, W = 32, 256, 256, 3, 56, 56
N_CORES = 8
BPC = B // N_CORES          # samples per core = 4
P = 128
NI = FIN // P               # 2
NO = FOUT // P              # 2
NT = H // 2                 # 28 row-pair tiles
KSQ = KK * KK
CFREE = KSQ * FOUT          # 2304 (kh kw o)
HALF = CFREE // 2
# r-tile stripes: 448-free matmuls run at their 209ns floor and 224-free
# ones at 116ns, both hiding the per-matmul LDWEIGHTS; 392-free (7,7,7,7)
# measures LDWEIGHTS-bound at 221ns -- keep (8,8,8,4)
STRIPES = ((0, 8), (8, 16), (16, 24), (24, 28))
F32 = mybir.dt.float32
F32R = mybir.dt.float32r
BF16 = mybir.dt.bfloat16


def build_program():
    nc = bacc.Bacc("TRN2", target_bir_lowering=False, debug=False,
                   num_devices=N_CORES)

    x_d = nc.dram_tensor("x", [BPC, FIN, H, W], BF16,
                         kind="ExternalInput").ap()
    mt_d = nc.dram_tensor("mt", [BPC, NI, P, CFREE], BF16,
                          kind="ExternalInput").ap()
    wt_d = nc.dram_tensor("wt", [NI, P, CFREE], BF16,
                          kind="ExternalInput").ap()
    b_d = nc.dram_tensor("bias", [FOUT], F32, kind="ExternalInput").ap()
    o_d = nc.dram_tensor("out", [BPC, FOUT, H, W], F32,
                         kind="ExternalOutput").ap()

    with tile.TileContext(nc) as tc, ExitStack() as ctx:
        consts = ctx.enter_context(tc.tile_pool(name="consts", bufs=1))
        mt_pool = ctx.enter_context(tc.tile_pool(name="mt_pool", bufs=2))
        mw_pool = ctx.enter_context(tc.tile_pool(name="mw_pool", bufs=6))
        u_pool = ctx.enter_context(tc.tile_pool(name="u_pool", bufs=6))
        t_pool = ctx.enter_context(tc.tile_pool(name="t_pool", bufs=2))
        xs_pool = ctx.enter_context(tc.tile_pool(name="xs_pool", bufs=2))
        v_pool = ctx.enter_context(tc.tile_pool(name="v_pool", bufs=18))
        m_pool = ctx.enter_context(tc.tile_pool(name="m_pool", bufs=16))
        ob_pool = ctx.enter_context(tc.tile_pool(name="ob_pool", bufs=2))
        of_pool = ctx.enter_context(tc.tile_pool(name="of_pool", bufs=2))
        acc_psum = ctx.enter_context(tc.tile_pool(name="acc_psum", bufs=8,
                                                  space="PSUM"))

        w_tiles = []
        for icc in range(NI):
            wt = consts.tile([P, CFREE], BF16, name=f"wt_{icc}", tag=f"w{icc}")
            w_tiles.append(wt)
        nc.sync.dma_start(out=w_tiles[0][:, :HALF], in_=wt_d[0][:, :HALF])

        bias_t = consts.tile([P, NO], F32, name="bias_t")
        # zero scratch row for the V0 t=0 edge case
        zeros = consts.tile([P, W], BF16, name="zeros")
        nc.vector.memset(zeros, 0.0)

        x_nat = x_d.rearrange("s (c p) h w -> s c p h w", p=P)
        o_nat = o_d.rearrange("s (c p) h w -> s c p (h w)", p=P)

        mt_tiles = {}
        xs_tiles = {}
        stat = {}     # (s, icc, oc, j) -> [stationary APs per kw]
        vt = {}       # (s, icc) -> [V_j tiles]

        def load_mt(s, icc):
            mt = mt_pool.tile([P, CFREE], BF16, name=f"mt_{s}_{icc}",
                              tag="mt")
            # bf16 full-tile DMA: 4608B descriptors (the descriptor-feed
            # rate, not bytes, paces the ring).  All mt on the sync ring:
            # at sample 0 this serializes ic1 BEHIND the critical ic0+wT
            # transfers so the PE's first work isn't starved by parallel
            # non-critical DMA traffic.
            nc.sync.dma_start(out=mt, in_=mt_d[s, icc])
            mt_tiles[(s, icc)] = mt

        def u_build(s, icc, oc):
            # mw = (weight*m) gathered for this oc: [128, (kh kw) * 128]
            mw = mw_pool.tile([P, KSQ * P], BF16,
                              name=f"mw_{s}_{icc}_{oc}", tag="mw")
            mtv = mt_tiles[(s, icc)].rearrange("p (k o) -> p k o", o=FOUT)
            wtv = w_tiles[icc].rearrange("p (k o) -> p k o", o=FOUT)
            mwv = mw.rearrange("p (k c) -> p k c", c=P)
            for k0, k1 in ((0, 4), (4, KSQ)):
                nc.vector.tensor_mul(
                    mwv[:, k0:k1],
                    mtv[:, k0:k1, oc * P:(oc + 1) * P],
                    wtv[:, k0:k1, oc * P:(oc + 1) * P])
            # U combos along kh: mw layout is kh-major [3, 3*128]
            mw3 = mw.rearrange("p (kh r) -> p kh r", kh=KK)
            tt = t_pool.tile([P, KK * P], BF16, name=f"t_{s}_{icc}_{oc}",
                             tag="tt")
            ut = u_pool.tile([P, 2 * KK * P], BF16,
                             name=f"u_{s}_{icc}_{oc}", tag="ut")
            nc.vector.tensor_add(tt, mw3[:, 0], mw3[:, 2])
            nc.vector.tensor_add(ut[:, :KK * P], tt, mw3[:, 1])
            nc.vector.tensor_sub(ut[:, KK * P:], tt, mw3[:, 1])
            bases = (mw[:, :KK * P], ut[:, :KK * P],
                     ut[:, KK * P:], mw[:, 2 * KK * P:])
            for j in range(4):
                stat[(s, icc, oc, j)] = [bases[j][:, kw * P:(kw + 1) * P]
                                         for kw in range(KK)]

        def load_x(s, icc):
            xs = xs_pool.tile([P, H * W], BF16, name=f"xs_{s}_{icc}",
                              tag="xs")
            nc.scalar.dma_start(out=xs, in_=x_nat[s, icc])
            xs_tiles[(s, icc)] = xs

        def v_build(s, icc):
            xsr = xs_tiles[(s, icc)].rearrange("p (t two w) -> p t two w",
                                               two=2, w=W)
            xse = xsr[:, :, 0, :]        # x[2t]
            xso = xsr[:, :, 1, :]        # x[2t+1]
            zrow = zeros[:, 0:W].rearrange("p (o w) -> p o w", o=1)
            vs = []
            for j in range(4):
                v = v_pool.tile([P, NT, W + 2], BF16,
                                name=f"v_{s}_{icc}_{j}", tag="v")
                nc.vector.memset(v[:, :, 0:1], 0.0)
                nc.vector.memset(v[:, :, W + 1:W + 2], 0.0)
                vs.append(v)
            # ops split at t=16 (stripe-2 boundary) so stripes 1-2
            # matmuls unblock on the first-half ops; V0 first since the
            # matmul j-loop consumes j=0 earliest
            HNT = 16
            # V0 = d0-d2 = x[2t-1]-x[2t+1]; t=0 row: 0 - x[1]
            nc.vector.tensor_sub(vs[0][:, 0:1, 1:W + 1],
                                 zrow, xso[:, 0:1, :])
            nc.vector.tensor_sub(vs[0][:, 1:HNT, 1:W + 1],
                                 xso[:, 0:HNT - 1, :], xso[:, 1:HNT, :])
            # V1 = d1+d2 = x[2t] + x[2t+1]; V2 = d2-d1
            nc.vector.tensor_add(vs[1][:, :HNT, 1:W + 1],
                                 xse[:, :HNT, :], xso[:, :HNT, :])
            nc.vector.tensor_sub(vs[2][:, :HNT, 1:W + 1],
                                 xso[:, :HNT, :], xse[:, :HNT, :])
            # V3 = d1-d3 = x[2t]-x[2t+2]
            nc.vector.tensor_sub(vs[3][:, 0:HNT, 1:W + 1],
                                 xse[:, 0:HNT, :], xse[:, 1:HNT + 1, :])
            # second halves (stripes 3-4)
            nc.vector.tensor_sub(vs[0][:, HNT:NT, 1:W + 1],
                                 xso[:, HNT - 1:NT - 1, :],
                                 xso[:, HNT:NT, :])
            nc.vector.tensor_add(vs[1][:, HNT:, 1:W + 1],
                                 xse[:, HNT:, :], xso[:, HNT:, :])
            nc.vector.tensor_sub(vs[2][:, HNT:, 1:W + 1],
                                 xso[:, HNT:, :], xse[:, HNT:, :])
            # V3 t=27 row: x[54] (d3 is the zero pad row)
            nc.vector.tensor_sub(vs[3][:, HNT:NT - 1, 1:W + 1],
                                 xse[:, HNT:NT - 1, :],
                                 xse[:, HNT + 1:NT, :])
            nc.vector.tensor_copy(vs[3][:, NT - 1:NT, 1:W + 1],
                                  xse[:, NT - 1:NT, :])
            vt[(s, icc)] = vs

        def compute_oc(s, oc):
            ob = ob_pool.tile([P, H, W], BF16, name=f"ob_{s}_{oc}", tag="ob")
            obr = ob.rearrange("p (t two) w -> p t two w", two=2)
            for (t0, t1) in STRIPES:
                stw = (t1 - t0) * W
                accs = [acc_psum.tile([P, stw], F32,
                                      name=f"acc_{s}_{oc}_{t0}_{j}",
                                      tag="acc")
                        for j in range(4)]
                for icc in range(NI):
                    for kw in range(KK):
                        first = (icc == 0 and kw == 0)
                        last = (icc == NI - 1 and kw == KK - 1)
                        for j in range(4):
                            rhs = vt[(s, icc)][j][:, t0:t1, kw:kw + W]
                            nc.tensor.matmul(
                                accs[j], stat[(s, icc, oc, j)][kw], rhs,
                                start=first, stop=last)
                # drains: 0.5 scale on M1/M2 folds the G-transform
                # scaling; bf16 M so the DVE inverse runs in 2x mode
                ms = []
                for j in range(4):
                    mj = m_pool.tile([P, stw], BF16,
                                     name=f"m_{s}_{oc}_{t0}_{j}", tag="m")
                    if j in (1, 2):
                        nc.scalar.mul(mj, accs[j], 0.5)
                    else:
                        nc.scalar.copy(mj, accs[j])
                    ms.append(mj)
                # inverse on DVE (all-bf16, packed -> 2x):
                #   even = (M0+M1) + M2 ; odd = (M1-M2) - M3
                tmp = t_pool.tile([P, stw], BF16, name=f"it_{s}_{oc}_{t0}",
                                  tag="it")
                msr = [m.rearrange("p (t w) -> p t w", w=W) for m in ms]
                tmpr = tmp.rearrange("p (t w) -> p t w", w=W)
                nc.vector.tensor_add(tmpr, msr[0], msr[1])
                nc.vector.tensor_add(obr[:, t0:t1, 0, :], tmpr, msr[2])
                nc.vector.tensor_sub(tmpr, msr[1], msr[2])
                nc.vector.tensor_sub(obr[:, t0:t1, 1, :], tmpr, msr[3])
            # bias + bf16->f32 cast on Act, then store (gpsimd ring),
            # chunked at stripe boundaries so the tail chain is short
            obf = ob.rearrange("p h w -> p (h w)")
            for (t0, t1) in STRIPES:
                lo, hi = t0 * 2 * W, t1 * 2 * W
                of = of_pool.tile([P, hi - lo], F32,
                                  name=f"of_{s}_{oc}_{t0}", tag="of")
                nc.scalar.add(of, obf[:, lo:hi], bias_t[:, oc:oc + 1])
                nc.gpsimd.dma_start(out=o_nat[s, oc][:, lo:hi], in_=of)

        # --- sample 0 prologue.  The DMA engines round-robin descriptors
        # across ALL queued transfers, so early non-critical triggers steal
        # bandwidth from the critical path.  Only ic0's mt/wT/x are queued
        # up front; everything else is emitted as DVE-ring dma_starts that
        # sit BEHIND the first mask-multiplies in the DVE queue and so fire
        # only once the critical data has landed. ---
        load_mt(0, 0)
        nc.sync.dma_start(out=w_tiles[0][:, HALF:], in_=wt_d[0][:, HALF:])
        load_x(0, 0)
        u_build(0, 0, 0)
        # deferred non-critical loads: a tiny Act op depending on the mt00
        # DMA blocks the Act queue, so these triggers fire only after the
        # critical data has landed (~6us) instead of competing with it
        gate = consts.tile([P, 1], BF16, name="gate")
        nc.scalar.copy(gate, mt_tiles[(0, 0)][:, 0:1])
        nc.scalar.dma_start(out=w_tiles[1][:, :HALF], in_=wt_d[1][:, :HALF])
        nc.scalar.dma_start(out=w_tiles[1][:, HALF:], in_=wt_d[1][:, HALF:])
        mt01 = mt_pool.tile([P, CFREE], BF16, name="mt_0_1", tag="mt")
        nc.scalar.dma_start(out=mt01, in_=mt_d[0, 1])
        mt_tiles[(0, 1)] = mt01
        xs01 = xs_pool.tile([P, H * W], BF16, name="xs_0_1", tag="xs")
        nc.scalar.dma_start(out=xs01, in_=x_nat[0, 1])
        xs_tiles[(0, 1)] = xs01
        v_build(0, 0)
        u_build(0, 1, 0)
        v_build(0, 1)
        u_build(0, 0, 1)
        u_build(0, 1, 1)
        # bias: 4B-per-descriptor storm -> idle gpsimd ring, out of the way
        nc.gpsimd.dma_start(out=bias_t,
                            in_=b_d.rearrange("(c p) -> p c", p=P))

        # --- software-pipelined emission: the next sample's ic0 prep sits
        # between this sample's oc0 and oc1 so its first stationaries and V
        # tiles are ready on the DVE before the sample boundary ---
        for s in range(BPC):
            compute_oc(s, 0)
            if s + 1 < BPC:
                load_mt(s + 1, 0)
                load_mt(s + 1, 1)
                u_build(s + 1, 0, 0)
                load_x(s + 1, 0)
                v_build(s + 1, 0)
            compute_oc(s, 1)
            if s + 1 < BPC:
                u_build(s + 1, 1, 0)
                load_x(s + 1, 1)
                v_build(s + 1, 1)
                u_build(s + 1, 0, 1)
                u_build(s + 1, 1, 1)

    nc.compile()
    return nc


def shard_inputs(x, m, weight, bias):
    x = np.ascontiguousarray(
        np.asarray(x, dtype=np.float32)).astype(ml_dtypes.bfloat16)
    m = np.asarray(m, dtype=np.float32)
    weight = np.asarray(weight, dtype=np.float32)
    bias = np.ascontiguousarray(np.asarray(bias, dtype=np.float32))
    mt = np.ascontiguousarray(m.transpose(0, 2, 3, 4, 1)).reshape(
        B, NI, P, CFREE).astype(ml_dtypes.bfloat16)
    wt = np.ascontiguousarray(weight.transpose(1, 2, 3, 0)).reshape(
        NI, P, CFREE).astype(ml_dtypes.bfloat16)
    in_maps = []
    for c in range(N_CORES):
        sl = slice(c * BPC, (c + 1) * BPC)
        in_maps.append({"x": x[sl], "mt": mt[sl], "wt": wt, "bias": bias})
    return in_maps


def kernel(x, m, weight, bias, _trace=False):
    nc = build_program()
    in_maps = shard_inputs(x, m, weight, bias)
    res = bass_utils.run_bass_kernel_spmd(
        nc, in_maps, core_ids=list(range(N_CORES)), trace=_trace
    )
    out = np.concatenate([res.results[c]["out"] for c in range(N_CORES)], axis=0)
    if _trace:
        kernel.last_results = res
    return out


# revision 43
# speedup vs baseline: 1.0098x; 1.0098x over previous
"""Trainium2 Bass kernel: per-sample masked conv2d via 1-D Winograd F(2,3).

out[b] = conv2d(x[b], weight * m[b], stride=1, pad=1) + bias

Data parallel over batch (32 -> 8 cores x 4).  The 3x3 conv is decomposed
with 1-D Winograd F(2,3) along H (kw stays a shifted-matmul accumulation):

  per output row-pair t (28 tiles):   d_a = xpad[2t+a],  a = 0..3
    V0 = d0-d2, V1 = d1+d2, V2 = d2-d1, V3 = d1-d3          (input transform)
    U0 = g0, U1 = (g0+g1+g2)/2, U2 = (g0-g1+g2)/2, U3 = g2  (weight transform)
    M_j = sum_{kw,i} U_j  (x)kw  V_j                         (4 j-matmuls)
    out[2t]   = M0 + M1 + M2 + bias
    out[2t+1] = M1 - M2 - M3 + bias                          (inverse)

This trades 18 matmul-passes per output pixel for 12 (1.5x less PE time).
The 1/2 scales on U1/U2 are folded into the Activation-engine PSUM drains.
M is drained to bf16 so the DVE inverse runs in 2x mode; a final Act pass
applies bias while converting bf16 -> f32.  The matmul path stays f32r via
zero-copy bitcasts (f32r is an f32-bits dtype tag enabling the full-rate
PE mode).  Stationary tiles are 128-col slices of the U tiles (f32r slice
LDWEIGHTS is ldw-opt compatible; bf16 is not).
"""

import sys
from contextlib import ExitStack

for _p in ("/opt/trn_rl_repo",):
    if _p not in sys.path:
        sys.path.append(_p)

import ml_dtypes
import numpy as np

import concourse.bass as bass
import concourse.mybir as mybir
import concourse.tile as tile
from concourse import bacc, bass_utils

B, FIN, FOUT, KK, H, W = 32, 256, 256, 3, 56, 56
N_CORES = 8
BPC = B // N_CORES          # samples per core = 4
P = 128
NI = FIN // P               # 2
NO = FOUT // P              # 2
NT = H // 2                 # 28 row-pair tiles
KSQ = KK * KK
CFREE = KSQ * FOUT          # 2304 (kh kw o)
HALF = CFREE // 2
# r-tile stripes: 448-free matmuls run at their 209ns floor and 224-free
# ones at 116ns, both hiding the per-matmul LDWEIGHTS; 392-free (7,7,7,7)
# measures LDWEIGHTS-bound at 221ns -- keep (8,8,8,4)
STRIPES = ((0, 8), (8, 16), (16, 24), (24, 28))
F32 = mybir.dt.float32
F32R = mybir.dt.float32r
BF16 = mybir.dt.bfloat16


def build_program():
    nc = bacc.Bacc("TRN2", target_bir_lowering=False, debug=False,
                   num_devices=N_CORES)

    x_d = nc.dram_tensor("x", [BPC, FIN, H, W], BF16,
                         kind="ExternalInput").ap()
    mt_d = nc.dram_tensor("mt", [BPC, NI, P, CFREE], BF16,
                          kind="ExternalInput").ap()
    wt_d = nc.dram_tensor("wt", [NI, P, CFREE], BF16,
                          kind="ExternalInput").ap()
    b_d = nc.dram_tensor("bias", [FOUT], F32, kind="ExternalInput").ap()
    o_d = nc.dram_tensor("out", [BPC, FOUT, H, W], F32,
                         kind="ExternalOutput").ap()

    with tile.TileContext(nc) as tc, ExitStack() as ctx:
        consts = ctx.enter_context(tc.tile_pool(name="consts", bufs=1))
        mt_pool = ctx.enter_context(tc.tile_pool(name="mt_pool", bufs=2))
        mw_pool = ctx.enter_context(tc.tile_pool(name="mw_pool", bufs=6))
        u_pool = ctx.enter_context(tc.tile_pool(name="u_pool", bufs=6))
        t_pool = ctx.enter_context(tc.tile_pool(name="t_pool", bufs=2))
        xs_pool = ctx.enter_context(tc.tile_pool(name="xs_pool", bufs=2))
        v_pool = ctx.enter_context(tc.tile_pool(name="v_pool", bufs=18))
        m_pool = ctx.enter_context(tc.tile_pool(name="m_pool", bufs=16))
        ob_pool = ctx.enter_context(tc.tile_pool(name="ob_pool", bufs=2))
        of_pool = ctx.enter_context(tc.tile_pool(name="of_pool", bufs=2))
        acc_psum = ctx.enter_context(tc.tile_pool(name="acc_psum", bufs=8,
                                                  space="PSUM"))

        w_tiles = []
        for icc in range(NI):
            wt = consts.tile([P, CFREE], BF16, name=f"wt_{icc}", tag=f"w{icc}")
            w_tiles.append(wt)
        nc.sync.dma_start(out=w_tiles[0][:, :HALF], in_=wt_d[0][:, :HALF])

        bias_t = consts.tile([P, NO], F32, name="bias_t")
        # zero scratch row for the V0 t=0 edge case
        zeros = consts.tile([P, W], BF16, name="zeros")
        nc.vector.memset(zeros, 0.0)

        x_nat = x_d.rearrange("s (c p) h w -> s c p h w", p=P)
        o_nat = o_d.rearrange("s (c p) h w -> s c p (h w)", p=P)

        mt_tiles = {}
        xs_tiles = {}
        stat = {}     # (s, icc, oc, j) -> [stationary APs per kw]
        vt = {}       # (s, icc) -> [V_j tiles]

        def load_mt(s, icc):
            mt = mt_pool.tile([P, CFREE], BF16, name=f"mt_{s}_{icc}",
                              tag="mt")
            # bf16 full-tile DMA: 4608B descriptors (the descriptor-feed
            # rate, not bytes, paces the ring).  All mt on the sync ring:
            # at sample 0 this serializes ic1 BEHIND the critical ic0+wT
            # transfers so the PE's first work isn't starved by parallel
            # non-critical DMA traffic.
            nc.sync.dma_start(out=mt, in_=mt_d[s, icc])
            mt_tiles[(s, icc)] = mt

        def u_build(s, icc, oc):
            # mw = (weight*m) gathered for this oc: [128, (kh kw) * 128]
            mw = mw_pool.tile([P, KSQ * P], BF16,
                              name=f"mw_{s}_{icc}_{oc}", tag="mw")
            mtv = mt_tiles[(s, icc)].rearrange("p (k o) -> p k o", o=FOUT)
            wtv = w_tiles[icc].rearrange("p (k o) -> p k o", o=FOUT)
            mwv = mw.rearrange("p (k c) -> p k c", c=P)
            for k0, k1 in ((0, 4), (4, KSQ)):
                nc.vector.tensor_mul(
                    mwv[:, k0:k1],
                    mtv[:, k0:k1, oc * P:(oc + 1) * P],
                    wtv[:, k0:k1, oc * P:(oc + 1) * P])
            # U combos along kh: mw layout is kh-major [3, 3*128]
            mw3 = mw.rearrange("p (kh r) -> p kh r", kh=KK)
            tt = t_pool.tile([P, KK * P], BF16, name=f"t_{s}_{icc}_{oc}",
                             tag="tt")
            ut = u_pool.tile([P, 2 * KK * P], BF16,
                             name=f"u_{s}_{icc}_{oc}", tag="ut")
            nc.vector.tensor_add(tt, mw3[:, 0], mw3[:, 2])
            nc.vector.tensor_add(ut[:, :KK * P], tt, mw3[:, 1])
            nc.vector.tensor_sub(ut[:, KK * P:], tt, mw3[:, 1])
            bases = (mw[:, :KK * P], ut[:, :KK * P],
                     ut[:, KK * P:], mw[:, 2 * KK * P:])
            for j in range(4):
                stat[(s, icc, oc, j)] = [bases[j][:, kw * P:(kw + 1) * P]
                                         for kw in range(KK)]

        def load_x(s, icc):
            xs = xs_pool.tile([P, H * W], BF16, name=f"xs_{s}_{icc}",
                              tag="xs")
            nc.scalar.dma_start(out=xs, in_=x_nat[s, icc])
            xs_tiles[(s, icc)] = xs

        def v_build(s, icc):
            xsr = xs_tiles[(s, icc)].rearrange("p (t two w) -> p t two w",
                                               two=2, w=W)
            xse = xsr[:, :, 0, :]        # x[2t]
            xso = xsr[:, :, 1, :]        # x[2t+1]
            zrow = zeros[:, 0:W].rearrange("p (o w) -> p o w", o=1)
            vs = []
            for j in range(4):
                v = v_pool.tile([P, NT, W + 2], BF16,
                                name=f"v_{s}_{icc}_{j}", tag="v")
                nc.vector.memset(v[:, :, 0:1], 0.0)
                nc.vector.memset(v[:, :, W + 1:W + 2], 0.0)
                vs.append(v)
            # ops split at t=16 (stripe-2 boundary) so stripes 1-2
            # matmuls unblock on the first-half ops; V0 first since the
            # matmul j-loop consumes j=0 earliest
            HNT = 16
            # V0 = d0-d2 = x[2t-1]-x[2t+1]; t=0 row: 0 - x[1]
            nc.vector.tensor_sub(vs[0][:, 0:1, 1:W + 1],
                                 zrow, xso[:, 0:1, :])
            nc.vector.tensor_sub(vs[0][:, 1:HNT, 1:W + 1],
                                 xso[:, 0:HNT - 1, :], xso[:, 1:HNT, :])
            # V1 = d1+d2 = x[2t] + x[2t+1]; V2 = d2-d1
            nc.vector.tensor_add(vs[1][:, :HNT, 1:W + 1],
                                 xse[:, :HNT, :], xso[:, :HNT, :])
            nc.vector.tensor_sub(vs[2][:, :HNT, 1:W + 1],
                                 xso[:, :HNT, :], xse[:, :HNT, :])
            # V3 = d1-d3 = x[2t]-x[2t+2]
            nc.vector.tensor_sub(vs[3][:, 0:HNT, 1:W + 1],
                                 xse[:, 0:HNT, :], xse[:, 1:HNT + 1, :])
            # second halves (stripes 3-4)
            nc.vector.tensor_sub(vs[0][:, HNT:NT, 1:W + 1],
                                 xso[:, HNT - 1:NT - 1, :],
                                 xso[:, HNT:NT, :])
            nc.vector.tensor_add(vs[1][:, HNT:, 1:W + 1],
                                 xse[:, HNT:, :], xso[:, HNT:, :])
            nc.vector.tensor_sub(vs[2][:, HNT:, 1:W + 1],
                                 xso[:, HNT:, :], xse[:, HNT:, :])
            # V3 t=27 row: x[54] (d3 is the zero pad row)
            nc.vector.tensor_sub(vs[3][:, HNT:NT - 1, 1:W + 1],
                                 xse[:, HNT:NT - 1, :],
                                 xse[:, HNT + 1:NT, :])
            nc.vector.tensor_copy(vs[3][:, NT - 1:NT, 1:W + 1],
                                  xse[:, NT - 1:NT, :])
            vt[(s, icc)] = vs

        def compute_oc(s, oc):
            ob = ob_pool.tile([P, H, W], BF16, name=f"ob_{s}_{oc}", tag="ob")
            obr = ob.rearrange("p (t two) w -> p t two w", two=2)
            for (t0, t1) in STRIPES:
                stw = (t1 - t0) * W
                accs = [acc_psum.tile([P, stw], F32,
                                      name=f"acc_{s}_{oc}_{t0}_{j}",
                                      tag="acc")
                        for j in range(4)]
                for icc in range(NI):
                    for kw in range(KK):
                        first = (icc == 0 and kw == 0)
                        last = (icc == NI - 1 and kw == KK - 1)
                        for j in range(4):
                            rhs = vt[(s, icc)][j][:, t0:t1, kw:kw + W]
                            nc.tensor.matmul(
                                accs[j], stat[(s, icc, oc, j)][kw], rhs,
                                start=first, stop=last)
                # drains: 0.5 scale on M1/M2 folds the G-transform
                # scaling; bf16 M so the DVE inverse runs in 2x mode
                ms = []
                for j in range(4):
                    mj = m_pool.tile([P, stw], BF16,
                                     name=f"m_{s}_{oc}_{t0}_{j}", tag="m")
                    if j in (1, 2):
                        nc.scalar.mul(mj, accs[j], 0.5)
                    else:
                        nc.scalar.copy(mj, accs[j])
                    ms.append(mj)
                # inverse on DVE (all-bf16, packed -> 2x):
                #   even = (M0+M1) + M2 ; odd = (M1-M2) - M3
                tmp = t_pool.tile([P, stw], BF16, name=f"it_{s}_{oc}_{t0}",
                                  tag="it")
                msr = [m.rearrange("p (t w) -> p t w", w=W) for m in ms]
                tmpr = tmp.rearrange("p (t w) -> p t w", w=W)
                nc.vector.tensor_add(tmpr, msr[0], msr[1])
                nc.vector.tensor_add(obr[:, t0:t1, 0, :], tmpr, msr[2])
                nc.vector.tensor_sub(tmpr, msr[1], msr[2])
                nc.vector.tensor_sub(obr[:, t0:t1, 1, :], tmpr, msr[3])
            # bias + bf16->f32 cast on Act, then store (gpsimd ring),
            # chunked at stripe boundaries so the tail chain is short
            obf = ob.rearrange("p h w -> p (h w)")
            for (t0, t1) in STRIPES:
                lo, hi = t0 * 2 * W, t1 * 2 * W
                of = of_pool.tile([P, hi - lo], F32,
                                  name=f"of_{s}_{oc}_{t0}", tag="of")
                nc.scalar.add(of, obf[:, lo:hi], bias_t[:, oc:oc + 1])
                nc.gpsimd.dma_start(out=o_nat[s, oc][:, lo:hi], in_=of)

        # --- sample 0 prologue.  The DMA engines round-robin descriptors
        # across ALL queued transfers, so early non-critical triggers steal
        # bandwidth from the critical path.  Only ic0's mt/wT/x are queued
        # up front; everything else is emitted as DVE-ring dma_starts that
        # sit BEHIND the first mask-multiplies in the DVE queue and so fire
        # only once the critical data has landed. ---
        load_mt(0, 0)
        nc.sync.dma_start(out=w_tiles[0][:, HALF:], in_=wt_d[0][:, HALF:])
        load_x(0, 0)
        u_build(0, 0, 0)
        # deferred non-critical loads: pinned past the critical inflow with
        # the scheduler's manual-wait API (the DMA engines round-robin all
        # queued transfers, so an early trigger steals critical bandwidth)
        with tc.tile_wait_until(0.008):
            nc.scalar.dma_start(out=w_tiles[1][:, :HALF],
                                in_=wt_d[1][:, :HALF])
            nc.scalar.dma_start(out=w_tiles[1][:, HALF:],
                                in_=wt_d[1][:, HALF:])
            mt01 = mt_pool.tile([P, CFREE], BF16, name="mt_0_1", tag="mt")
            nc.scalar.dma_start(out=mt01, in_=mt_d[0, 1])
            mt_tiles[(0, 1)] = mt01
            xs01 = xs_pool.tile([P, H * W], BF16, name="xs_0_1", tag="xs")
            nc.scalar.dma_start(out=xs01, in_=x_nat[0, 1])
            xs_tiles[(0, 1)] = xs01
        v_build(0, 0)
        u_build(0, 1, 0)
        v_build(0, 1)
        u_build(0, 0, 1)
        u_build(0, 1, 1)
        # bias: 4B-per-descriptor storm -> idle gpsimd ring, out of the way
        nc.gpsimd.dma_start(out=bias_t,
                            in_=b_d.rearrange("(c p) -> p c", p=P))

        # --- software-pipelined emission: the next sample's ic0 prep sits
        # between this sample's oc0 and oc1 so its first stationaries and V
        # tiles are ready on the DVE before the sample boundary ---
        for s in range(BPC):
            compute_oc(s, 0)
            if s + 1 < BPC:
                load_mt(s + 1, 0)
                load_mt(s + 1, 1)
                u_build(s + 1, 0, 0)
                load_x(s + 1, 0)
                v_build(s + 1, 0)
            compute_oc(s, 1)
            if s + 1 < BPC:
                u_build(s + 1, 1, 0)
                load_x(s + 1, 1)
                v_build(s + 1, 1)
                u_build(s + 1, 0, 1)
                u_build(s + 1, 1, 1)

    nc.compile()
    return nc


def shard_inputs(x, m, weight, bias):
    x = np.ascontiguousarray(
        np.asarray(x, dtype=np.float32)).astype(ml_dtypes.bfloat16)
    m = np.asarray(m, dtype=np.float32)
    weight = np.asarray(weight, dtype=np.float32)
    bias = np.ascontiguousarray(np.asarray(bias, dtype=np.float32))
    mt = np.ascontiguousarray(m.transpose(0, 2, 3, 4, 1)).reshape(
        B, NI, P, CFREE).astype(ml_dtypes.bfloat16)
    wt = np.ascontiguousarray(weight.transpose(1, 2, 3, 0)).reshape(
        NI, P, CFREE).astype(ml_dtypes.bfloat16)
    in_maps = []
    for c in range(N_CORES):
        sl = slice(c * BPC, (c + 1) * BPC)
        in_maps.append({"x": x[sl], "mt": mt[sl], "wt": wt, "bias": bias})
    return in_maps


def kernel(x, m, weight, bias, _trace=False):
    nc = build_program()
    in_maps = shard_inputs(x, m, weight, bias)
    res = bass_utils.run_bass_kernel_spmd(
        nc, in_maps, core_ids=list(range(N_CORES)), trace=_trace
    )
    out = np.concatenate([res.results[c]["out"] for c in range(N_CORES)], axis=0)
    if _trace:
        kernel.last_results = res
    return out
